# revision 24
# baseline (speedup 1.0000x reference)
"""Bahdanau multi-head attention on 8 Trainium2 NeuronCores.

Sharding: 8 shards = (batch B=4) x (query-half Lq=128). Each core owns ALL
heads for its 128 query rows, so the W0 output projection is fully local.

Math: tanh(q+k) is expanded in a 6-term sine series at odd harmonics of a
base frequency (tanh(x) ~ sum_m c_m sin((2m-1) w x)), which turns the
Bahdanau score contraction into PE matmuls over (sin/cos feature, dk)
pairs instead of a 16.7M-element ScalarE tanh:

  scores[q,k] = sum_d vp_d tanh(qh_d + kh_d)
             ~= sum_m c_m sum_d vp_d [sin_m(q)cos_m(k) + cos_m(q)sin_m(k)]

Per-core device algorithm (L=256 keys, H=8 heads, DK=64):
  1. Duplicated projections on PE (fp16, FWL): psum qh2[h] = [128=(2 x dk),
     128 q] per head (dk rows duplicated so sin/cos share one tile), ACT
     Copy+per-head-bias moves them to SBUF fp16. Same for kh2 (256 cols).
  2. Seed features via ACT Sin with per-partition phase {0, pi/2}:
     F1 = [sin(w x); cos(w x)] (args <= 3.06 < pi, inside the Sin table's
     valid range). ACT Square + DVE tensor_scalar give C2 = 2cos(2 w x).
  3. Chebyshev ladder on DVE (fp16 2x): F3 = F1*(C2 +- 1),
     F{m+2} = C2*Fm - F{m-2} produces odd harmonics {1,3,5,7,9,11}.
     vp folds into the q-side seed, c_m applied per harmonic.
  4. Scores: 6 matmuls per head accumulate [128 q, 256 k] f32 in PSUM with
     queries on partitions - the natural softmax orientation, no transpose.
  5. Softmax without max-subtraction (|scores| <= ~6 so exp is safe):
     ACT Exp straight from PSUM, DVE rowsum/reciprocal/scale.
  6. attn transposed via plain matmul against identity (faster than
     transpose-mode), attn @ vh on PE, W0 projection, +b0, DMA out.
"""

import numpy as np

B, L, D, H, DK = 4, 256, 512, 8, 64
NCORES = 8
QT = 128          # query rows per core
NCH = D // 128    # 4 chunks of 128 along D
NEG = -30.0       # masked-score penalty (exp(-30) ~ 1e-13)

OM_HALF = 0.300
HARMONICS = (1, 3, 5, 7, 9)
M = len(HARMONICS)


def _fit_coeffs():
    om = np.array([m * OM_HALF for m in HARMONICS])
    x = np.linspace(-9.9, 9.9, 4001)
    wgt = np.exp(-x * x / 4.0) + 3e-3
    A = np.sin(np.outer(x, om))
    w = np.sqrt(wgt)[:, None]
    c, *_ = np.linalg.lstsq(A * w, np.tanh(x) * w[:, 0], rcond=None)
    return c.astype(np.float64)


_C = _fit_coeffs()

_compiled = {}


def _build_nc(masked):
    import concourse.mybir as mybir
    import concourse.tile as tile
    from concourse import bacc

    f32 = mybir.dt.float32
    f16 = mybir.dt.float16
    AF = mybir.ActivationFunctionType
    ALU = mybir.AluOpType
    AX = mybir.AxisListType

    nc = bacc.Bacc(
        "TRN2",
        target_bir_lowering=False,
        debug=False,
        enable_asserts=False,
        num_devices=NCORES,
    )

    # consolidated input blobs - one DMA each (HWDGE charges ~625ns per
    # DMA regardless of size, so 35 small DMAs would serialize ~22us):
    # blob32: [sbq 8 | sbk 8 | vpdup 8 | ppc 6 | bvB 512 | b0B 512]
    # blobA:  [Wqd 4x1024 | qT 4x128 | ident 128]
    # blobB:  [Wkd 4x1024 | kT 4x256]
    # blobC:  [Wv 4x512 | vT 4x256 | W0 4x512]
    N32 = 1054
    NA = 4 * 1024 + 4 * QT + 128 + 128 + 2 * D
    NB = 4 * 1024 + 4 * L
    NC = 2048 + 4 * L + 2048
    blob32 = nc.dram_tensor("blob32", [128, N32], f32, kind="ExternalInput").ap()
    blobA = nc.dram_tensor("blobA", [128, NA], f16, kind="ExternalInput").ap()
    blobB = nc.dram_tensor("blobB", [128, NB], f16, kind="ExternalInput").ap()
    blobC = nc.dram_tensor("blobC", [128, NC], f16, kind="ExternalInput").ap()
    if masked:
        pen = nc.dram_tensor("pen", [QT, L], f32, kind="ExternalInput").ap()
    out = nc.dram_tensor("out", [QT, D], f32, kind="ExternalOutput").ap()

    NQ = H * QT   # 1024 q-side feature cols
    NK = H * L    # 2048 k-side feature cols

    with tile.TileContext(nc) as tc:
        with tc.tile_pool(name="const", bufs=1) as cp:
            t32 = cp.tile([128, N32], f32, tag="t32")
            nc.sync.dma_start(t32[:], blob32[:])
            tA = cp.tile([128, NA], f16, tag="tA")
            nc.sync.dma_start(tA[:], blobA[:])
            tB = cp.tile([128, NB], f16, tag="tB")
            nc.sync.dma_start(tB[:], blobB[:])
            tC = cp.tile([128, NC], f16, tag="tC")
            nc.sync.dma_start(tC[:], blobC[:])
            if masked:
                pen_t = cp.tile([QT, L], f32, tag="pen")
                nc.sync.dma_start(pen_t[:], pen[:])

            sbq_t = t32[:, 0:H]
            sbk_t = t32[:, H : 2 * H]
            vpdup_t = t32[:, 2 * H : 3 * H]
            ppc_t = t32[:, 3 * H : 3 * H + 6]
            bvB_t = t32[:, 30 : 30 + D]
            b0B_t = t32[:, 30 + D : 30 + 2 * D]
            Wqd_t = [tA[:, c * 1024 : (c + 1) * 1024] for c in range(NCH)]
            qT_t = [tA[:, 4096 + c * QT : 4096 + (c + 1) * QT] for c in range(NCH)]
            ident_t = tA[:, 4096 + 4 * QT : 4096 + 4 * QT + 128]
            Wkd_t = [tB[:, c * 1024 : (c + 1) * 1024] for c in range(NCH)]
            kT_t = [tB[:, 4096 + c * L : 4096 + (c + 1) * L] for c in range(NCH)]
            Wv_t = [tC[:, c * 512 : (c + 1) * 512] for c in range(NCH)]
            vT_t = [tC[:, 2048 + c * L : 2048 + (c + 1) * L] for c in range(NCH)]
            W0_t = [tC[:, 3072 + c * 512 : 3072 + (c + 1) * 512] for c in range(NCH)]
            o_ones = 4096 + 4 * QT + 128
            ones_r = tA[0:1, o_ones : o_ones + 128]
            bv_r = tA[0:1, o_ones + 128 : o_ones + 128 + D]
            b0_r = tA[0:1, o_ones + 128 + D : o_ones + 128 + 2 * D]

            vh16 = [cp.tile([128, D], f16, tag=f"vh{j}", name=f"vh{j}") for j in range(2)]
            F1q = cp.tile([128, NQ], f16, tag="F1q")
            F1k = cp.tile([128, NK], f16, tag="F1k")
            Gq = [cp.tile([128, NQ], f16, tag=f"Gq{i}", name=f"Gq{i}") for i in range(M)]
            Hq = [cp.tile([128, NQ], f16, tag=f"Hq{i}", name=f"Hq{i}") for i in range(M)]
            Kk = [None] * M
            Kk[0] = F1k
            Kk[1] = cp.tile([128, NK], f16, tag="Kk1", name="Kk1")
            for i in range(2, M):
                Kk[i] = cp.tile([128, NK], f16, tag=f"Kk{i}", name=f"Kk{i}")
            tmpq = [cp.tile([128, NQ], f16, tag=f"tq{i}", name=f"tq{i}") for i in range(M - 2)]
            tmpk = [cp.tile([128, NK], f16, tag=f"tk{i}", name=f"tk{i}") for i in range(M - 2)]

            # ---- projections fused with seed Sin; q side fully first so
            # the q ladder starts while blobB/blobC are still in flight ----
            with (
                tc.tile_pool(name="psq", bufs=2, space="PSUM") as psqp,
                tc.tile_pool(name="psk", bufs=2, space="PSUM") as pskp,
                tc.tile_pool(name="psv", bufs=2, space="PSUM") as psvp,
            ):
                for h in range(H):
                    hs = slice(h * 128, (h + 1) * 128)
                    psq = psqp.tile([128, QT], f32, tag="psq", name=f"psq{h}")
                    for c in range(NCH):
                        nc.tensor.matmul(
                            psq[:], lhsT=Wqd_t[c][:, hs], rhs=qT_t[c],
                            start=(c == 0), stop=(c == NCH - 1),
                        )
                    nc.scalar.activation(
                        F1q[:, hs], psq[:], AF.Sin,
                        bias=sbq_t[:, h : h + 1], scale=OM_HALF,
                    )

                # q-side multipliers + full q ladder (DVE) - overlaps the
                # k projection and remaining input DMA
                def mults(F1, rng, n, sfx, o):
                    mp = ppc_t[:, o : o + 1]
                    ap = ppc_t[:, o + 1 : o + 2]
                    pm1 = ppc_t[:, o + 2 : o + 3]
                    SQ = cp.tile([128, n], f16, tag=f"SQ{sfx}", name=f"SQ{sfx}")
                    nc.scalar.activation(SQ[:], F1[:], AF.Square)
                    C2 = cp.tile([128, n], f16, tag=f"C2{sfx}", name=f"C2{sfx}")
                    nc.vector.tensor_scalar(C2[:], SQ[:], mp, ap, ALU.mult, ALU.add)
                    M3 = cp.tile([128, n], f16, tag=f"M3{sfx}", name=f"M3{sfx}")
                    nc.vector.tensor_scalar(M3[:], C2[:], pm1, None, ALU.add)
                    return C2, M3

                for h in range(H):
                    hs = slice(h * 128, (h + 1) * 128)
                    nc.vector.tensor_scalar_mul(
                        Gq[0][:, hs], F1q[:, hs], vpdup_t[:, h : h + 1]
                    )
                C2q, M3q = mults(F1q, None, NQ, "q", 0)
                nc.vector.tensor_scalar_mul(Hq[0][:], Gq[0][:], float(_C[0]))
                nc.vector.tensor_mul(Gq[1][:], Gq[0][:], M3q[:])
                nc.vector.tensor_scalar_mul(Hq[1][:], Gq[1][:], float(_C[1]))
                for i in range(2, M):
                    nc.vector.tensor_mul(tmpq[i - 2][:], C2q[:], Gq[i - 1][:])
                    nc.vector.tensor_sub(Gq[i][:], tmpq[i - 2][:], Gq[i - 2][:])
                    nc.vector.tensor_scalar_mul(Hq[i][:], Gq[i][:], float(_C[i]))

                # k projection + seeds
                for h in range(H):
                    hs = slice(h * 128, (h + 1) * 128)
                    ks = slice(h * L, (h + 1) * L)
                    psk = pskp.tile([128, L], f32, tag="psk", name=f"psk{h}")
                    for c in range(NCH):
                        nc.tensor.matmul(
                            psk[:], lhsT=Wkd_t[c][:, hs], rhs=kT_t[c],
                            start=(c == 0), stop=(c == NCH - 1),
                        )
                    nc.scalar.activation(
                        F1k[:, ks], psk[:], AF.Sin,
                        bias=sbk_t[:, h : h + 1], scale=OM_HALF,
                    )
                C2k, M3k = mults(F1k, None, NK, "k", 3)

                # v projection with bv folded in via a ones-row matmul
                for j in range(2):
                    js = slice(j * 128, (j + 1) * 128)
                    psv = psvp.tile([128, D], f32, tag="psv", name=f"psv{j}")
                    for c in range(NCH):
                        nc.tensor.matmul(
                            psv[:], lhsT=vT_t[c][:, js], rhs=Wv_t[c],
                            start=(c == 0), stop=False,
                        )
                    nc.tensor.matmul(
                        psv[:], lhsT=ones_r, rhs=bv_r, start=False, stop=True,
                    )
                    nc.scalar.activation(vh16[j][:], psv[:], AF.Copy)

            # ---- k ladder + scores + softmax, interleaved ----
            with (
                tc.tile_pool(name="sc", bufs=4, space="PSUM") as scp,
                tc.tile_pool(name="tr", bufs=2, space="PSUM") as trp,
                tc.tile_pool(name="av", bufs=1, space="PSUM") as avp,
                tc.tile_pool(name="smp", bufs=3) as smp,
                tc.tile_pool(name="ops", bufs=1, space="PSUM") as opsp,
            ):
                # two heads per PSUM bank: head h lives in sc_ps[h//2]
                # cols [(h%2)*L, (h%2+1)*L). start=True clears has_written
                # bank-wide, so only the bank's FIRST matmul (h even, i=0)
                # sets it; the odd head's first write lands on cleared bits
                # and overwrites correctly with start=False.
                sc_ps = [
                    scp.tile([128, 2 * L], f32, tag="sc", name=f"sc{hp}")
                    for hp in range(H // 2)
                ]

                def sc_ap(h):
                    return sc_ps[h // 2][:, (h % 2) * L : (h % 2 + 1) * L]

                def kstep(i, half):
                    hk = slice(half * NK // 2, (half + 1) * NK // 2)
                    if i == 1:
                        nc.vector.tensor_mul(Kk[1][:, hk], F1k[:, hk], M3k[:, hk])
                    elif i >= 2:
                        nc.vector.tensor_mul(tmpk[i - 2][:, hk], C2k[:, hk], Kk[i - 1][:, hk])
                        nc.vector.tensor_sub(Kk[i][:, hk], tmpk[i - 2][:, hk], Kk[i - 2][:, hk])
                    for h in range(4 * half, 4 * half + 4):
                        nc.tensor.matmul(
                            sc_ap(h),
                            lhsT=Hq[i][:, h * 128 : (h + 1) * 128],
                            rhs=Kk[i][:, h * L : (h + 1) * L],
                            start=(i == 0 and h % 2 == 0),
                            stop=(i == M - 1),
                        )

                aoT = [cp.tile([128, QT], f16, tag=f"aoT{c}", name=f"aoT{c}") for c in range(NCH)]
                ops = opsp.tile([QT, D], f32, tag="ops")

                def softmax_head(h):
                    if masked:
                        spen = smp.tile([QT, L], f32, tag="spen")
                        nc.vector.tensor_add(spen[:], sc_ap(h), pen_t[:])
                        p = smp.tile([QT, L], f16, tag="p")
                        nc.scalar.activation(p[:], spen[:], AF.Exp)
                    else:
                        p = smp.tile([QT, L], f16, tag="p")
                        nc.scalar.activation(p[:], sc_ap(h), AF.Exp)
                    rs = smp.tile([QT, 1], f32, tag="rs")
                    nc.vector.tensor_reduce(rs[:], p[:], axis=AX.X, op=ALU.add)
                    rcp = smp.tile([QT, 1], f32, tag="rcp")
                    nc.vector.reciprocal_approx_fast(rcp[:], rs[:])
                    attn = smp.tile([QT, L], f16, tag="attn")
                    nc.vector.tensor_scalar_mul(attn[:], p[:], rcp[:])
                    av = avp.tile([64, QT], f32, tag="av", name=f"av{h}")
                    for j in range(2):
                        js = slice(j * 128, (j + 1) * 128)
                        tr = trp.tile([128, QT], f32, tag="tr")
                        nc.tensor.matmul(
                            tr[:], lhsT=attn[:, js], rhs=ident_t,
                            start=True, stop=True,
                        )
                        attnT = smp.tile([128, QT], f16, tag="attnT")
                        if j == 0:
                            nc.vector.tensor_copy(attnT[:], tr[:])
                        else:
                            nc.scalar.activation(attnT[:], tr[:], AF.Copy)
                        nc.tensor.matmul(
                            av[:], lhsT=vh16[j][:, h * 64 : (h + 1) * 64],
                            rhs=attnT[:], start=(j == 0), stop=(j == 1),
                        )
                    hh = h % 2
                    nc.scalar.activation(
                        aoT[h // 2][hh * 64 : (hh + 1) * 64, :], av[:], AF.Copy
                    )
                    if hh == 1:
                        c = h // 2
                        nc.tensor.matmul(
                            ops[:], lhsT=aoT[c][:], rhs=W0_t[c],
                            start=(c == 0), stop=False,
                        )
                    if h == H - 1:
                        nc.tensor.matmul(
                            ops[:], lhsT=ones_r, rhs=b0_r, start=False, stop=True,
                        )

                for i in range(M):
                    kstep(i, 0)
                for i in range(M):
                    kstep(i, 1)
                    if i >= 1:
                        softmax_head(i - 1)
                for h in range(4, 8):
                    softmax_head(h)

                y = smp.tile([QT, D], f32, tag="y")
                nc.scalar.activation(y[:], ops[:], AF.Copy)
                nc.sync.dma_start(out[:], y[:])

    nc.compile()
    return nc


def _dup_cols(W):
    # [D, D] -> [D, H*128] with cols h*128 + r*64 + j = W[:, h*64 + j]
    idx = np.arange(H * 128)
    src = (idx // 128) * 64 + (idx % 64)
    return np.ascontiguousarray(W[:, src])


def _dup_part(v):
    # [D] -> [128, H] with [p, h] = v[h*64 + p%64]
    p = np.arange(128) % 64
    return np.ascontiguousarray(v[p[:, None] + np.arange(H)[None, :] * 64])


def build_in_maps(q, k, v, mask, Wq, bq, Wk, bk, Wv, bv, vp, W0, b0):
    q = np.asarray(q, np.float32)
    k = np.asarray(k, np.float32)
    v = np.asarray(v, np.float32)
    mask = np.asarray(mask)
    vp = np.asarray(vp, np.float32).reshape(H, DK)

    masked = not bool(np.all(mask != 0))

    # blob32: [sbq 8 | sbk 8 | vpdup 8 | ppc 6 | bvB 512 | b0B 512]
    blob32 = np.zeros((128, 1054), np.float32)
    phq = np.zeros((128, 1), np.float32)
    phq[64:] = np.pi / 2                 # q: [sin; cos]
    phk = np.zeros((128, 1), np.float32)
    phk[:64] = np.pi / 2                 # k: [cos; sin]
    blob32[:, 0:H] = OM_HALF * _dup_part(np.asarray(bq, np.float32)) + phq
    blob32[:, H : 2 * H] = OM_HALF * _dup_part(np.asarray(bk, np.float32)) + phk
    blob32[:, 2 * H : 3 * H] = vp.T[np.arange(128) % 64]
    ppc = blob32[:, 3 * H : 3 * H + 6]
    # q side ([sin; cos]): mp {-4, 4}, ap {2, -2}, pm1 {1, -1}
    ppc[:64, 0], ppc[64:, 0] = -4.0, 4.0
    ppc[:64, 1], ppc[64:, 1] = 2.0, -2.0
    ppc[:64, 2], ppc[64:, 2] = 1.0, -1.0
    # k side ([cos; sin]): halves swapped
    ppc[:64, 3], ppc[64:, 3] = 4.0, -4.0
    ppc[:64, 4], ppc[64:, 4] = -2.0, 2.0
    ppc[:64, 5], ppc[64:, 5] = -1.0, 1.0
    blob32[:, 30:542] = np.asarray(bv, np.float32)[None, :]
    blob32[:, 542:1054] = np.asarray(b0, np.float32)[None, :]

    def chunks(W, n):
        # [512, n] -> [128, 4*n] chunk-major
        return W.reshape(NCH, 128, n).transpose(1, 0, 2).reshape(128, NCH * n)

    Wqd = _dup_cols(np.asarray(Wq, np.float32)).astype(np.float16)
    Wkd = _dup_cols(np.asarray(Wk, np.float32)).astype(np.float16)
    Wv16 = np.asarray(Wv, np.float32).astype(np.float16)
    W016 = np.asarray(W0, np.float32).astype(np.float16)

    # blobA: [Wqd 4x1024 | qT 4x128 | ident 128 | ones 128 | bv 512 | b0 512]
    blobA_shared = np.zeros((128, 4096 + 4 * QT + 128 + 128 + 2 * D), np.float16)
    blobA_shared[:, :4096] = chunks(Wqd, 1024)
    o = 4096 + 4 * QT
    blobA_shared[:, o : o + 128] = np.eye(128, dtype=np.float16)
    blobA_shared[0, o + 128 : o + 256] = 1.0
    blobA_shared[0, o + 256 : o + 256 + D] = np.asarray(bv, np.float32).astype(np.float16)
    blobA_shared[0, o + 256 + D : o + 256 + 2 * D] = np.asarray(b0, np.float32).astype(np.float16)
    # blobB: [Wkd 4x1024 | kT 4x256]
    blobB_shared = np.empty((128, 4096 + 4 * L), np.float16)
    blobB_shared[:, :4096] = chunks(Wkd, 1024)
    # blobC: [Wv 4x512 | vT 4x256 | W0 4x512]
    blobC_shared = np.empty((128, 2048 + 4 * L + 2048), np.float16)
    blobC_shared[:, :2048] = chunks(Wv16, 512)
    blobC_shared[:, 2048 + 4 * L :] = chunks(W016, 512)

    in_maps = []
    for c in range(NCORES):
        b, half = c // 2, c % 2
        rows = slice(half * QT, (half + 1) * QT)
        bA = blobA_shared.copy()
        bA[:, 4096 : 4096 + 4 * QT] = chunks(
            np.ascontiguousarray(q[b, rows, :].T).astype(np.float16), QT
        )
        bB = blobB_shared.copy()
        bB[:, 4096:] = chunks(
            np.ascontiguousarray(k[b].T).astype(np.float16), L
        )
        bC = blobC_shared.copy()
        bC[:, 2048 : 2048 + 4 * L] = chunks(
            np.ascontiguousarray(v[b].T).astype(np.float16), L
        )
        m = dict(blob32=blob32, blobA=bA, blobB=bB, blobC=bC)
        if masked:
            m["pen"] = np.ascontiguousarray(
                np.where(mask[b, rows, :] == 0, NEG, 0.0).astype(np.float32)
            )
        in_maps.append(m)
    return in_maps, masked


def kernel(q, k, v, mask, Wq, bq, Wk, bk, Wv, bv, vp, W0, b0):
    in_maps, masked = build_in_maps(
        q, k, v, mask, Wq, bq, Wk, bk, Wv, bv, vp, W0, b0
    )
    if masked not in _compiled:
        _compiled[masked] = _build_nc(masked)
    from concourse.bass_utils import run_bass_kernel_spmd

    res = run_bass_kernel_spmd(
        _compiled[masked], in_maps, core_ids=list(range(NCORES))
    )
    outf = np.zeros((B, L, D), np.float32)
    for c, r in enumerate(res.results):
        b, half = c // 2, c % 2
        outf[b, half * QT : (half + 1) * QT, :] = r["out"]
    return outf
